# revision 1
# baseline (speedup 1.0000x reference)
# Multi-head attention (B=4, L=2048, D=512, H=8, dh=64) on 8 trn2 cores.
# Sharding: core c -> batch b = c//2, head-group hg = c%2 (4 heads, 256 out
# channels per core). Full per-core computation is done on-chip:
#   - PE-transpose q, k, W slices (fp32)
#   - QT/KT/V projections as float32r matmuls accumulating in PSUM
#   - per (q-block 512, head-pair): S^T = K_h @ Q_h^T row-packed pair matmuls
#     -> fused scale+exp on ScalarE (PSUM -> SBUF)
#     -> PV matmul with a ones-column appended to V (row-sums for free)
#     -> PE-transpose back to natural layout, fused normalize+residual on DVE
import sys

import numpy as np

sys.path.insert(0, "/opt/trn_rl_repo")

L = 2048
D = 512
NH = 4          # heads per core
DH = 64
DHG = NH * DH   # 256 output channels per core
NLT = L // 128  # 16 row tiles
NCI = D // 128  # 4 feature chunks
QB = 512        # q block
NQB = L // QB   # 4
INV_SCALE = 1.0 / float(np.sqrt(D))

_cache = {}


def _build():
    import concourse.bacc as bacc
    import concourse.mybir as mybir
    import concourse.tile as tile
    from concourse.masks import make_identity

    f32 = mybir.dt.float32
    f32r = mybir.dt.float32r
    EXP = mybir.ActivationFunctionType.Exp
    MUL = mybir.AluOpType.mult
    ADD = mybir.AluOpType.add

    nc = bacc.Bacc("TRN2", target_bir_lowering=False, debug=False, num_devices=8)
    q_d = nc.dram_tensor("q", [L, D], f32, kind="ExternalInput").ap()
    k_d = nc.dram_tensor("k", [L, D], f32, kind="ExternalInput").ap()
    wq_d = nc.dram_tensor("wq", [DHG, D], f32, kind="ExternalInput").ap()
    wk_d = nc.dram_tensor("wk", [DHG, D], f32, kind="ExternalInput").ap()
    wv_d = nc.dram_tensor("wv", [DHG, D], f32, kind="ExternalInput").ap()
    qres_d = nc.dram_tensor("qres", [L, DHG], f32, kind="ExternalInput").ap()
    o_d = nc.dram_tensor("o", [L, DHG], f32, kind="ExternalOutput").ap()

    def r(ap):
        return ap.bitcast(f32r)

    with tile.TileContext(nc) as tc:
        with (
            tc.tile_pool(name="const", bufs=1) as const_pool,
            tc.tile_pool(name="big", bufs=1) as big_pool,
            tc.tile_pool(name="vpool", bufs=1) as v_pool,
            tc.tile_pool(name="nat", bufs=6) as nat_pool,
            tc.tile_pool(name="wT", bufs=1) as wT_pool,
            tc.tile_pool(name="pstage", bufs=6) as p_pool,
            tc.tile_pool(name="outsb", bufs=8) as outsb_pool,
            tc.tile_pool(name="qresp", bufs=8) as qres_pool,
            tc.tile_pool(name="misc", bufs=3) as misc_pool,
            tc.tile_pool(name="ps_s", bufs=3, space="PSUM") as ps_s,
            tc.tile_pool(name="ps_pv", bufs=2, space="PSUM") as ps_pv,
        ):
            identity = const_pool.tile([128, 128], f32, name="identity")
            make_identity(nc, identity)

            # transposed activations / weights, as single wide tensors
            # qT[:, ci*L + l] = q[l, ci*128 + p]
            qT = big_pool.tile([128, NCI * L], f32r, name="qT")
            kT = big_pool.tile([128, NCI * L], f32r, name="kT")
            # QT[:, p*L + l] = Q[l, p*128 + row]   (2 chunks of 128 features)
            QT = big_pool.tile([128, 2 * L], f32r, name="QT")
            KT = big_pool.tile([128, 2 * L], f32r, name="KT")
            # wT[:, ci*DHG + o] = W[o, ci*128 + p]
            wqT = wT_pool.tile([128, NCI * DHG], f32r, name="wqT")
            wkT = wT_pool.tile([128, NCI * DHG], f32r, name="wkT")
            wvT = wT_pool.tile([128, NCI * DHG], f32r, name="wvT")

            # V with ones columns: V_all[kt] = [128, 4*65], head h in
            # cols [h*65, h*65+64), ones at col h*65+64
            V_all = [
                v_pool.tile([128, NH * (DH + 1)], f32r, name=f"Vall{kt}")
                for kt in range(NLT)
            ]
            for kt in range(NLT):
                ones_view = (
                    V_all[kt]
                    .bitcast(f32)
                    .rearrange("p (h x) -> p h x", h=NH)[:, :, DH : DH + 1]
                )
                nc.vector.memset(ones_view, 1.0)

            def transpose_into(dst, dst_off, src_tile, n_ci, ci_stride):
                # PE-transpose n_ci blocks of [128,128] from src_tile into a
                # single PSUM batch tile, then one strided DVE copy into
                # dst[:, dst_off + ci*ci_stride : +128] for each ci.
                assert n_ci == NCI
                tpb = ps_s.tile([128, 512], f32, tag="s", name="tpb")
                for ci in range(n_ci):
                    nc.tensor.transpose(
                        tpb[:, ci * 128 : (ci + 1) * 128],
                        src_tile[:, ci * 128 : (ci + 1) * 128],
                        identity,
                    )
                dst_view = dst.rearrange("p (c x) -> p c x", c=n_ci)[
                    :, :, dst_off : dst_off + 128
                ]
                nc.vector.tensor_copy(
                    dst_view, tpb.rearrange("p (c x) -> p c x", c=n_ci)
                )

            # ---- W loads + transposes (emitted on demand, see below)
            def w_t(wd, wT_t, oc):
                wn = nat_pool.tile([128, D], f32, tag="nat", name="wn")
                nc.sync.dma_start(out=wn, in_=wd[oc * 128 : (oc + 1) * 128, :])
                transpose_into(wT_t, oc * 128, wn, NCI, DHG)

            # ---------- emission helpers ----------
            def k_transpose(lt):
                kn = nat_pool.tile([128, D], f32, tag="nat", name="kn")
                nc.sync.dma_start(out=kn, in_=k_d[lt * 128 : (lt + 1) * 128, :])
                transpose_into(kT, lt * 128, kn, NCI, L)

            def kt_proj(lb):
                for p in range(2):
                    ps = ps_s.tile([128, 2 * QB], f32, tag="s", name="kps")
                    for ci in range(NCI):
                        nc.tensor.matmul(
                            ps[:, 0:QB],
                            lhsT=(wkT[:, ci * DHG + p * 128 : ci * DHG + (p + 1) * 128]),
                            rhs=(kT[:, ci * L + lb * QB : ci * L + (lb + 1) * QB]),
                            start=(ci == 0),
                            stop=(ci == NCI - 1),
                        )
                    nc.vector.tensor_copy(
                        KT[:, p * L + lb * QB : p * L + (lb + 1) * QB], ps[:, 0:QB]
                    )

            def v_proj(kt):
                ps = ps_s.tile([128, 2 * QB], f32, tag="s", name="vps")
                for ci in range(NCI):
                    nc.tensor.matmul(
                        ps[:, 0:DHG],
                        lhsT=(kT[:, ci * L + kt * 128 : ci * L + (kt + 1) * 128]),
                        rhs=(wvT[:, ci * DHG : (ci + 1) * DHG]),
                        start=(ci == 0),
                        stop=(ci == NCI - 1),
                    )
                nc.vector.tensor_copy(
                    V_all[kt].rearrange("p (h x) -> p h x", h=NH)[:, :, 0:DH],
                    ps[:, 0:DHG].rearrange("p (h x) -> p h x", h=NH),
                )

            def q_side_pieces(qb):
                pieces = []
                for j in range(4):
                    def tp_piece(j=j):
                        lt = qb * 4 + j
                        qn = nat_pool.tile([128, D], f32, tag="nat", name="qn")
                        nc.sync.dma_start(
                            out=qn, in_=q_d[lt * 128 : (lt + 1) * 128, :]
                        )
                        transpose_into(qT, lt * 128, qn, NCI, L)
                    pieces.append(tp_piece)
                for p in range(2):
                    def pj_piece(p=p):
                        ps = ps_s.tile([128, 2 * QB], f32, tag="s", name="qps")
                        for ci in range(NCI):
                            nc.tensor.matmul(
                                ps[:, 0:QB],
                                lhsT=(
                                    wqT[:, ci * DHG + p * 128 : ci * DHG + (p + 1) * 128]
                                ),
                                rhs=(qT[:, ci * L + qb * QB : ci * L + (qb + 1) * QB]),
                                start=(ci == 0),
                                stop=(ci == NCI - 1),
                            )
                        nc.vector.tensor_copy(
                            QT[:, p * L + qb * QB : p * L + (qb + 1) * QB], ps[:, 0:QB]
                        )
                    pieces.append(pj_piece)
                return pieces

            def q_side(qb):
                # q transposes + QT projection for this q block
                for j in range(4):
                    lt = qb * 4 + j
                    qn = nat_pool.tile([128, D], f32, tag="nat", name="qn")
                    nc.sync.dma_start(out=qn, in_=q_d[lt * 128 : (lt + 1) * 128, :])
                    transpose_into(qT, lt * 128, qn, NCI, L)
                for p in range(2):
                    ps = ps_s.tile([128, 2 * QB], f32, tag="s", name="qps")
                    for ci in range(NCI):
                        nc.tensor.matmul(
                            ps[:, 0:QB],
                            lhsT=(wqT[:, ci * DHG + p * 128 : ci * DHG + (p + 1) * 128]),
                            rhs=(qT[:, ci * L + qb * QB : ci * L + (qb + 1) * QB]),
                            start=(ci == 0),
                            stop=(ci == NCI - 1),
                        )
                    nc.vector.tensor_copy(
                        QT[:, p * L + qb * QB : p * L + (qb + 1) * QB], ps[:, 0:QB]
                    )

            def attn_groups(
                qb, pair, pv, kt_groups, acc_first=0, acc_last=NLT - 1, filler=None
            ):
                # S^T matmuls (row-packed head pair) -> exp -> PV accumulation
                for g0, g1 in kt_groups:
                    if filler:
                        for _ in range(min(filler[1], len(filler[0]))):
                            filler[0].pop(0)()
                    n = g1 - g0
                    sA = ps_s.tile([128, 2 * QB], f32, tag="s", name="sA")
                    sB = ps_s.tile([128, 2 * QB], f32, tag="s", name="sB")
                    for j, kt in enumerate(range(g0, g1)):
                        for hh, st in ((0, sA), (1, sB)):
                            nc.tensor.matmul(
                                st[:, j * QB : (j + 1) * QB],
                                lhsT=KT[
                                    hh * 64 : (hh + 1) * 64,
                                    pair * L + kt * 128 : pair * L + (kt + 1) * 128,
                                ],
                                rhs=QT[
                                    hh * 64 : (hh + 1) * 64,
                                    pair * L + qb * QB : pair * L + (qb + 1) * QB,
                                ],
                                start=True,
                                stop=True,
                                tile_position=(hh * 64, 0),
                            )
                    pA = p_pool.tile([128, 2 * QB], f32r, tag="p", name="pA")
                    pB = p_pool.tile([128, 2 * QB], f32r, tag="p", name="pB")
                    nc.scalar.activation(
                        pA[:, 0 : n * QB], sA[:, 0 : n * QB], EXP, scale=INV_SCALE
                    )
                    nc.scalar.activation(
                        pB[:, 0 : n * QB], sB[:, 0 : n * QB], EXP, scale=INV_SCALE
                    )
                    for j, kt in enumerate(range(g0, g1)):
                        for hh, pt in ((0, pA), (1, pB)):
                            nc.tensor.matmul(
                                pv[hh][0 : DH + 1, :],
                                lhsT=V_all[kt][
                                    :,
                                    (2 * pair + hh) * (DH + 1) : (2 * pair + hh + 1)
                                    * (DH + 1),
                                ],
                                rhs=pt[:, j * QB : (j + 1) * QB],
                                start=(kt == acc_first),
                                stop=(kt == acc_last),
                            )

            def out_chain_sb(pair, osb_pair, out_t, qres_t):
                # transpose back from [d+1, q] to [q, d+1], normalize + residual
                for hh in range(2):
                    h = 2 * pair + hh
                    osb = osb_pair[hh]
                    ot = ps_pv.tile([128, 512], f32, tag="pv", name="otps")
                    for j in range(4):
                        nc.tensor.transpose(
                            ot[:, j * 128 : j * 128 + DH + 1],
                            osb[0 : DH + 1, j * 128 : (j + 1) * 128],
                            identity[0 : DH + 1, 0 : DH + 1],
                        )
                    recip = misc_pool.tile([128, 4], f32, tag="recip", name="recip")
                    for j in range(4):
                        nc.vector.reciprocal(
                            recip[:, j : j + 1], ot[:, j * 128 + DH : j * 128 + DH + 1]
                        )
                    for j in range(4):
                        nc.vector.scalar_tensor_tensor(
                            out=out_t[j][:, h * DH : (h + 1) * DH],
                            in0=ot[:, j * 128 : j * 128 + DH],
                            scalar=recip[:, j : j + 1],
                            in1=qres_t[j][:, h * DH : (h + 1) * DH],
                            op0=MUL,
                            op1=ADD,
                        )

            def qb_buffers(qb):
                qres_t, out_t = [], []
                for j in range(4):
                    lt = qb * 4 + j
                    qr = qres_pool.tile([128, DHG], f32, tag="qr", name="qr")
                    nc.sync.dma_start(out=qr, in_=qres_d[lt * 128 : (lt + 1) * 128, :])
                    qres_t.append(qr)
                    out_t.append(outsb_pool.tile([128, DHG], f32, tag="ot", name="ot_sb"))
                return qres_t, out_t

            def dma_out(qb, out_t):
                for j in range(4):
                    lt = qb * 4 + j
                    nc.sync.dma_start(
                        out=o_d[lt * 128 : (lt + 1) * 128, :], in_=out_t[j]
                    )

            # ---------- emission ----------
            # qb0: interleave k-side production with BOTH pairs' first sweep.
            # PV partials are flushed to SBUF accumulators per key-block so
            # the two PSUM pv slots can rotate between the two pairs.
            def k_prod_pieces(lb):
                pieces = []
                for j in range(4):
                    pieces.append(lambda lt=lb * 4 + j: k_transpose(lt))
                pieces.append(lambda lb=lb: kt_proj(lb))
                for kt in range(lb * 4, lb * 4 + 4):
                    pieces.append(lambda kt=kt: v_proj(kt))
                return pieces

            for _wd, _wT in ((wq_d, wqT), (wk_d, wkT), (wv_d, wvT)):
                for oc in range(2):
                    w_t(_wd, _wT, oc)
            q_side(0)
            qres_t, out_t = qb_buffers(0)
            osb_acc = [
                [
                    misc_pool.tile([128, 512], f32, tag="osb", name=f"osb{pair}{hh}", bufs=6)
                    for hh in range(2)
                ]
                for pair in range(2)
            ]
            for j in range(4):
                k_transpose(j)
            kt_proj(0)
            for kt in range(4):
                v_proj(kt)
            for lb in range(NQB):
                if lb < NQB - 1:
                    fill_list = k_prod_pieces(lb + 1)
                else:
                    fill_list = q_side_pieces(1)
                for pair in range(2):
                    pv = [
                        ps_pv.tile([128, 512], f32, tag="pv", name=f"pvacc{hh}")
                        for hh in range(2)
                    ]
                    attn_groups(
                        0,
                        pair,
                        pv,
                        [(lb * 4, lb * 4 + 2), (lb * 4 + 2, lb * 4 + 4)],
                        acc_first=lb * 4,
                        acc_last=lb * 4 + 3,
                        filler=(fill_list, 1),
                    )
                    for hh in range(2):
                        if lb == 0:
                            nc.vector.tensor_copy(
                                osb_acc[pair][hh][0 : DH + 1, :], pv[hh][0 : DH + 1, :]
                            )
                        else:
                            nc.vector.tensor_add(
                                osb_acc[pair][hh][0 : DH + 1, :],
                                osb_acc[pair][hh][0 : DH + 1, :],
                                pv[hh][0 : DH + 1, :],
                            )
                for piece in fill_list:
                    piece()
                fill_list.clear()
            for pair in range(2):
                out_chain_sb(pair, osb_acc[pair], out_t, qres_t)
            dma_out(0, out_t)

            # remaining q blocks
            groups16 = [(i, i + 2) for i in range(0, NLT, 2)]
            for qb in range(1, NQB):
                qres_t, out_t = qb_buffers(qb)
                for pair in range(2):
                    pv = [
                        ps_pv.tile([128, 512], f32, tag="pv", name=f"pvacc{hh}")
                        for hh in range(2)
                    ]
                    filler = (
                        (q_side_pieces(qb + 1), 1)
                        if (pair == 1 and qb < NQB - 1)
                        else None
                    )
                    attn_groups(qb, pair, pv, groups16, filler=filler)
                    osb_pair = []
                    for hh in range(2):
                        osb = misc_pool.tile([128, 512], f32, tag="osb", name="osb", bufs=6)
                        nc.vector.tensor_copy(osb[0 : DH + 1, :], pv[hh][0 : DH + 1, :])
                        osb_pair.append(osb)
                    out_chain_sb(pair, osb_pair, out_t, qres_t)
                dma_out(qb, out_t)

    nc.compile()
    return nc


def kernel(query, keys, Wq, Wk, Wv):
    from concourse.bass_utils import run_bass_kernel_spmd

    if "nc" not in _cache:
        _cache["nc"] = _build()
    nc = _cache["nc"]

    query = np.asarray(query, dtype=np.float32)
    keys = np.asarray(keys, dtype=np.float32)
    Wq = np.asarray(Wq, dtype=np.float32)
    Wk = np.asarray(Wk, dtype=np.float32)
    Wv = np.asarray(Wv, dtype=np.float32)
    B = query.shape[0]
    assert query.shape == (4, L, D) and keys.shape == (4, L, D)
    assert Wq.shape == (D, D) and Wk.shape == (D, D) and Wv.shape == (D, D)

    in_maps = []
    for c in range(8):
        b, hg = c // 2, c % 2
        sl = slice(hg * DHG, (hg + 1) * DHG)
        in_maps.append(
            {
                "q": np.ascontiguousarray(query[b]),
                "k": np.ascontiguousarray(keys[b]),
                "wq": np.ascontiguousarray(Wq[sl]),
                "wk": np.ascontiguousarray(Wk[sl]),
                "wv": np.ascontiguousarray(Wv[sl]),
                "qres": np.ascontiguousarray(query[b][:, sl]),
            }
        )
    res = run_bass_kernel_spmd(nc, in_maps, list(range(8)), **_cache.get("run_kwargs", {}))
    _cache["last_result"] = res
    out = np.empty((B, L, D), np.float32)
    for c in range(8):
        b, hg = c // 2, c % 2
        out[b][:, hg * DHG : (hg + 1) * DHG] = res.results[c]["o"]
    return out

